# revision 1
# baseline (speedup 1.0000x reference)
"""Trainium2 Bass kernel for the GRU-GCN cell (nn_GRUCell).

Sharding: 8 NeuronCores, node-parallel (128 nodes/core, all 32 batches).
All matmuls fp16 operands with fp32 PSUM accumulation; layernorm in fp32.
Cross-core: AllGather of layernormed embeddings (transposed) and of the
z*state tensor between the gate and candidate GCNs.
"""

import os
import sys

sys.path.insert(0, "/opt/trn_rl_repo")
import numpy as np

B, N, D = 32, 1024, 64
DI = DO = 64
C = DI + DO  # 128
OG, OU = 2 * DO, DO  # 128, 64
NCORES = 8
NL = N // NCORES  # 128 nodes per core
NG = NL // 4  # 32 col-pack groups of 4 nodes
EPS = 1e-12

_CACHE = {}
LAST_RESULT = None  # test harness reads timing info from here


def _np_reference(x, state, node_emb, time_emb, gate_w, gate_b, gate_gamma,
                  gate_beta, upd_w, upd_b, upd_gamma, upd_beta):
    """Plain numpy fallback (general layernorm parameters)."""

    def _ln(v, g, b2):
        mu = v.mean(-1, keepdims=True)
        var = ((v - mu) ** 2).mean(-1, keepdims=True)
        return (v - mu) / np.sqrt(var + EPS) * g + b2

    def _gcn(xg, w_pool, b_pool, g, b2):
        emb = _ln(node_emb[None] + time_emb[:, None], g, b2)
        logits = np.einsum("bnd,bmd->bnm", emb, emb, optimize=True)
        a = np.exp(logits - logits.max(-1, keepdims=True))
        a /= a.sum(-1, keepdims=True)
        xg2 = np.einsum("bnm,bmc->bnc", a, xg, optimize=True)
        w = np.einsum("nd,dkio->nkio", node_emb, w_pool, optimize=True)
        bias = time_emb @ b_pool
        return (np.einsum("bni,nio->bno", xg, w[:, 0], optimize=True)
                + np.einsum("bni,nio->bno", xg2, w[:, 1], optimize=True)
                + bias[:, None, :])

    inp = np.concatenate([x, state], -1)
    zr = 1.0 / (1.0 + np.exp(-_gcn(inp, gate_w, gate_b, gate_gamma, gate_beta)))
    z, r = zr[..., :DO], zr[..., DO:]
    cand = np.concatenate([x, z * state], -1)
    hc = np.tanh(_gcn(cand, upd_w, upd_b, upd_gamma, upd_beta))
    return (r * state + (1.0 - r) * hc).astype(np.float32)


def _install_prof_shim():
    """Provide antenv.axon_hooks if absent so trace=True can NTFF-profile."""
    import types

    if "antenv.axon_hooks" in sys.modules:
        return
    try:
        from trn_agent_boot.trn_boot import _ntff_profile_via_ctypes

        hook = _ntff_profile_via_ctypes("/opt/axon/libaxon_pjrt.so")
    except Exception:
        hook = None
    mod = types.ModuleType("antenv.axon_hooks")
    mod.get_axon_ntff_profile_hook = lambda: hook

    def _set(h):
        mod.get_axon_ntff_profile_hook = lambda: h

    mod.set_axon_ntff_profile_hook = _set
    sys.modules["antenv.axon_hooks"] = mod
    try:
        import antenv

        antenv.axon_hooks = mod
    except Exception:
        pass


def _build(phases=4):
    import concourse.bacc as bacc
    import concourse.mybir as mybir
    from concourse.tile import TileContext
    from concourse.masks import make_identity

    F16 = mybir.dt.float16
    F32 = mybir.dt.float32
    AF = mybir.ActivationFunctionType
    ALU = mybir.AluOpType

    nc = bacc.Bacc()

    class _Stop(Exception):
        pass

    def pin(name, shape, dt=F16):
        return nc.declare_dram_parameter(name, shape, dt, isOutput=False)

    ne_f32 = pin("ne_f32", [NL, D], F32)      # node_emb local rows (LN input)
    neT16 = pin("neT16", [D, NL])             # node_embT local (w-gen rhs)
    te_f32 = pin("te_f32", [B, D], F32)       # time_emb (LN input)
    teT16 = pin("teT16", [D, B])              # bias matmul lhsT
    x16 = pin("x16", [B, N, DI])
    st16 = pin("st16", [B, N, DO])
    xT16 = pin("xT16", [DI, B, NL])           # c-major local x
    stT16 = pin("stT16", [DO, B, NL])
    st_loc = pin("st_loc", [B, NL, DO], F32)  # natural local state (fp32)
    pg16 = pin("pg16", [2, OG, D, C])         # gate_w permuted (k,o,d,i)
    pu16 = pin("pu16", [2, OU, D, C])
    gb16 = pin("gb16", [D, OG])
    ub16 = pin("ub16", [D, OU])
    h_out = nc.declare_dram_parameter("h_out", [128, NG * DO], F32, isOutput=True)

    with TileContext(nc) as tc:
        with (
            tc.tile_pool(name="const", bufs=1) as cpool,
            tc.tile_pool(name="big", bufs=1) as big,
            tc.tile_pool(name="stage", bufs=2) as stg,
            tc.tile_pool(name="dram", bufs=1, space="DRAM") as dram,
        ):
          try:
              # ---------- constants / persistent tiles ----------
              ones_row = cpool.tile([1, 128], F32, tag="ones_row")
              nc.gpsimd.memset(ones_row[:], 1.0)
              ones16r = cpool.tile([1, 128], F16, tag="ones16r")
              nc.gpsimd.memset(ones16r[:], 1.0)
              ones_col16 = cpool.tile([128, 1], F16, tag="ones_col16")
              nc.gpsimd.memset(ones_col16[:], 1.0)
              ident16 = cpool.tile([128, 128], F16, tag="ident16")
              make_identity(nc, ident16[:])
              neg64_col = cpool.tile([128, 1], F32, tag="neg64_col")
              nc.gpsimd.memset(neg64_col[:], -64.0)

              ne_sb = cpool.tile([NL, D], F32, tag="ne_sb")
              nc.gpsimd.dma_start(ne_sb[:], ne_f32[:])
              te_row = cpool.tile([1, B * D], F32, tag="te_row")
              nc.gpsimd.dma_start(
                  te_row[:].rearrange("p (b d) -> p b d", d=D),
                  te_f32[:].unsqueeze(0),
              )
              neT_sb = cpool.tile([D, NL], F16, tag="neT_sb")
              nc.gpsimd.dma_start(neT_sb[:], neT16[:])
              teT_sb = cpool.tile([D, B], F16, tag="teT_sb")
              nc.gpsimd.dma_start(teT_sb[:], teT16[:])
              gb_sb = cpool.tile([D, OG], F16, tag="gb_sb")
              nc.gpsimd.dma_start(gb_sb[:], gb16[:])
              ub_sb = cpool.tile([D, OU], F16, tag="ub_sb")
              nc.gpsimd.dma_start(ub_sb[:], ub16[:])

              embT_loc = big.tile([D, B * NL], F16, tag="embT_loc")
              xg2T = big.tile([C, B * NL], F16, tag="xg2T")
              xg2uT = big.tile([C, B * NL], F16, tag="xg2uT")
              inpT_cm = big.tile([C, B * NL], F16, tag="inpT_cm")
              candT = big.tile([C, B * NL], F16, tag="candT")
              zr_sb = big.tile([128, NG * OG], F16, tag="zr_sb")  # [128, 4096]
              state_grp = big.tile([128, NG * DO], F32, tag="state_grp")
              zs_grp = big.tile([128, NG * DO], F16, tag="zs_grp")
              hc_sb = big.tile([128, NG * DO], F16, tag="hc_sb")
              h_sb = big.tile([128, NG * DO], F32, tag="h_sb")
              t1_sb = big.tile([128, NG * DO], F32, tag="t1_sb")
              sinv_sb = big.tile([1, B * NL], F16, tag="sinv_sb")
              biasg_rep = big.tile([128, OG], F32, tag="biasg_rep")
              biasu_rep = big.tile([128, OU], F32, tag="biasu_rep")
              wslab = big.tile([C, 2 * OG * NL], F16, tag="wslab")  # 8.4MB

              nc.gpsimd.memset(h_sb[:], 0.0)

              # c-major inputs (one DMA each)
              nc.gpsimd.dma_start(inpT_cm[0:DI, :], xT16[:].rearrange("d b n -> d (b n)"))
              nc.gpsimd.dma_start(inpT_cm[DI:C, :], stT16[:].rearrange("d b n -> d (b n)"))
              nc.gpsimd.dma_start(candT[0:DI, :], xT16[:].rearrange("d b n -> d (b n)"))

              # state group tiles [32*jj + b | g*64 + o]
              for jj in range(4):
                  nc.gpsimd.dma_start(
                      state_grp[32 * jj : 32 * jj + 32, :]
                      .rearrange("b (g o) -> b g o", o=DO),
                      st_loc[:].rearrange("b (g jj) o -> b g jj o", jj=4)[:, :, jj, :],
                  )

              # DRAM scratch
              d_embT_in = dram.tile([D, B * NL], F16, tag="d_embT_in")
              d_embT_out = dram.tile([NCORES, D, B * NL], F16, tag="d_embT_out")
              d_exp = dram.tile([B, 128, 8 * NL], F16, tag="d_exp")
              d_zs_in = dram.tile([NL, B * DO], F16, tag="d_zs_in")
              d_zs_out = dram.tile([NCORES, NL, B * DO], F16, tag="d_zs_out")

              # ---------- bias tiles: bias = time_emb @ pool_b, replicated ----
              with tc.tile_pool(name="psb", bufs=1, space="PSUM") as psb:
                  ps_bg = psb.tile([B, OG], F32, tag="ps_bg")
                  nc.tensor.matmul(ps_bg[:], teT_sb[:], gb_sb[:], start=True, stop=True)
                  bg_row = stg.tile([B, OG], F32, tag="bg_row")
                  nc.vector.tensor_copy(bg_row[:], ps_bg[:])
                  ps_bu = psb.tile([B, OU], F32, tag="ps_bu")
                  nc.tensor.matmul(ps_bu[:], teT_sb[:], ub_sb[:], start=True, stop=True)
                  bu_row = stg.tile([B, OU], F32, tag="bu_row")
                  nc.vector.tensor_copy(bu_row[:], ps_bu[:])
                  for jj in range(4):
                      nc.gpsimd.dma_start(biasg_rep[32 * jj : 32 * jj + 32, :], bg_row[:])
                      nc.gpsimd.dma_start(biasu_rep[32 * jj : 32 * jj + 32, :], bu_row[:])

              # ---------- phase E: layernormed embeddings, transposed ----------
              with (
                  tc.tile_pool(name="embu", bufs=1) as embu,
                  tc.tile_pool(name="pse", bufs=2, space="PSUM") as pse,
              ):
                  u_all = embu.tile([NL, B * D], F32, tag="u_all")
                  for b in range(B):
                      ps_trep = pse.tile([NL, D], F32, tag="trep")
                      nc.tensor.matmul(
                          ps_trep[:], ones_row[:],
                          te_row[:, b * D : (b + 1) * D],
                          start=True, stop=True,
                      )
                      nc.vector.tensor_add(
                          u_all[:, b * D : (b + 1) * D], ne_sb[:], ps_trep[:]
                      )
                  scr_all = embu.tile([NL, B * D], F32, tag="scr_all")
                  nc.vector.tensor_mul(scr_all[:], u_all[:], u_all[:])
                  sm_all = stg.tile([NL, B], F32, tag="sm_all")
                  nc.vector.reduce_sum(
                      sm_all[:],
                      u_all[:].rearrange("p (b d) -> p b d", d=D),
                      axis=mybir.AxisListType.X,
                  )
                  sq_all = stg.tile([NL, B], F32, tag="sq_all")
                  nc.vector.reduce_sum(
                      sq_all[:],
                      scr_all[:].rearrange("p (b d) -> p b d", d=D),
                      axis=mybir.AxisListType.X,
                  )
                  mu_all = stg.tile([NL, B], F32, tag="mu_all")
                  nc.vector.tensor_scalar_mul(mu_all[:], sm_all[:], 1.0 / D)
                  musq = stg.tile([NL, B], F32, tag="musq")
                  nc.vector.tensor_mul(musq[:], mu_all[:], mu_all[:])
                  var_all = stg.tile([NL, B], F32, tag="var_all")
                  nc.vector.tensor_scalar_mul(var_all[:], sq_all[:], 1.0 / D)
                  nc.vector.tensor_sub(var_all[:], var_all[:], musq[:])
                  nc.vector.tensor_scalar_add(var_all[:], var_all[:], EPS)
                  sd_all = stg.tile([NL, B], F32, tag="sd_all")
                  nc.scalar.sqrt(sd_all[:], var_all[:])
                  rstd_all = stg.tile([NL, B], F32, tag="rstd_all")
                  nc.vector.reciprocal(rstd_all[:], sd_all[:])
                  for b in range(B):
                      embn = stg.tile([NL, D], F16, tag="embn")
                      nc.vector.tensor_scalar(
                          out=embn[:], in0=u_all[:, b * D : (b + 1) * D],
                          scalar1=mu_all[:, b : b + 1],
                          scalar2=rstd_all[:, b : b + 1],
                          op0=ALU.subtract, op1=ALU.mult,
                      )
                      ps_tr = pse.tile([D, NL], F16, tag="ps_tr")
                      nc.tensor.transpose(ps_tr[:], embn[:], ident16[:])
                      nc.vector.tensor_copy(
                          embT_loc[:, b * NL : (b + 1) * NL], ps_tr[:]
                      )
              if phases < 0.3:
                  raise _Stop()
              nc.gpsimd.dma_start(d_embT_in[:], embT_loc[:])
              nc.gpsimd.collective_compute(
                  "AllGather",
                  ALU.bypass,
                  replica_groups=[list(range(NCORES))],
                  ins=[d_embT_in.opt()],
                  outs=[d_embT_out.opt()],
              )

              # ---------- w-gen (gate pool); overlaps the AllGather ----------
              def wgen(pool_dram, n_o):
                  with (
                      tc.tile_pool(name="psw", bufs=4, space="PSUM") as psw,
                      tc.tile_pool(name="pwstg", bufs=1) as pwstg,
                  ):
                      ohs = max(1, n_o // 64)
                      osz = n_o // ohs
                      for k in range(2):
                          for oh in range(ohs):
                              pw = pwstg.tile([D, osz * C], F16, tag="pw")
                              nc.gpsimd.dma_start(
                                  pw[:],
                                  pool_dram[k, oh * osz : (oh + 1) * osz]
                                  .rearrange("o d i -> d o i"),
                              )
                              for oo in range(osz):
                                  o = oh * osz + oo
                                  ps_w = psw.tile([C, NL], F32, tag="ps_w")
                                  nc.tensor.matmul(
                                      ps_w[:], pw[:, oo * C : (oo + 1) * C],
                                      neT_sb[:], start=True, stop=True,
                                  )
                                  col = (k * n_o + o) * NL
                                  if o % 2 == 0:
                                      nc.vector.tensor_copy(
                                          wslab[:, col : col + NL], ps_w[:]
                                      )
                                  else:
                                      nc.scalar.activation(
                                          wslab[:, col : col + NL], ps_w[:], AF.Copy
                                      )

              if phases < 0.5:
                  raise _Stop()
              wgen(pg16, OG)
              if phases < 0.7:
                  raise _Stop()

              # ---------- gate phase ----------
              with tc.tile_pool(name="psg", bufs=2, space="PSUM") as psg:
                  for b in range(B):
                      it_b = stg.tile([128, 8, C], F16, tag="it_b")
                      nc.gpsimd.dma_start(
                          it_b[:, :, 0:DI],
                          x16[b].rearrange("(q m) d -> m q d", m=128),
                      )
                      nc.gpsimd.dma_start(
                          it_b[:, :, DI:C],
                          st16[b].rearrange("(q m) d -> m q d", m=128),
                      )
                      embT_b = stg.tile([D, N], F16, tag="embT_b")
                      nc.gpsimd.dma_start(
                          embT_b[:].rearrange("d (q n) -> d q n", n=NL),
                          d_embT_out[:, :, b * NL : (b + 1) * NL]
                          .rearrange("q d n -> d q n"),
                      )
                      exp_b = stg.tile([128, 8 * NL], F16, tag="exp_b")
                      ps_xg2 = psg.tile([C, NL], F32, tag="xg2")
                      ps_s = psg.tile([1, NL], F32, tag="s")
                      for q in range(8):
                          ps_l = psg.tile([128, NL], F32, tag="ltile")
                          nc.tensor.matmul(
                              ps_l[:],
                              embT_b[:, q * 128 : (q + 1) * 128],
                              embT_loc[:, b * NL : (b + 1) * NL],
                              start=True, stop=True,
                          )
                          et = exp_b[:, q * NL : (q + 1) * NL]
                          nc.scalar.activation(et, ps_l[:], AF.Exp, bias=neg64_col[:])
                          nc.tensor.matmul(
                              ps_s[:], ones_col16[:], et,
                              start=(q == 0), stop=(q == 7),
                          )
                          nc.tensor.matmul(
                              ps_xg2[:], it_b[:, q, :], et,
                              start=(q == 0), stop=(q == 7),
                          )
                      nc.gpsimd.dma_start(d_exp[b], exp_b[:])
                      with nc.allow_low_precision("softmax scale in fp16"):
                          nc.vector.reciprocal(
                              sinv_sb[:, b * NL : (b + 1) * NL], ps_s[:]
                          )
                      ps_rep = psg.tile([128, NL], F32, tag="rep")
                      nc.tensor.matmul(
                          ps_rep[:], ones16r[:],
                          sinv_sb[:, b * NL : (b + 1) * NL],
                          start=True, stop=True,
                      )
                      rep_sb = stg.tile([128, NL], F32, tag="rep_sb")
                      nc.vector.tensor_copy(rep_sb[:], ps_rep[:])
                      nc.vector.tensor_mul(
                          xg2T[:, b * NL : (b + 1) * NL], ps_xg2[:], rep_sb[:]
                      )

              # ---------- gate out-mm ----------
              if phases < 2:
                  raise _Stop()
              wview = wslab[:].rearrange("c (k o n) -> c k o n", k=2, o=OG)
              with tc.tile_pool(name="pso", bufs=3, space="PSUM") as pso:
                  for g in range(NG):
                      ps_og = pso.tile([128, OG], F32, tag="og")
                      for jj in range(4):
                          n_ = g * 4 + jj
                          for ki, src in ((0, inpT_cm), (1, xg2T)):
                              lhs = src[:].rearrange("c (b n) -> c n b", n=NL)[:, n_, :]
                              rhs = wview[:, ki, :, n_]
                              nc.tensor.matmul(
                                  ps_og[32 * jj : 32 * jj + 32, :],
                                  lhs, rhs,
                                  start=(ki == 0), stop=(ki == 1),
                                  tile_position=(0, 32 * jj),
                              )
                      zt = stg.tile([128, OG], F32, tag="zt")
                      nc.vector.tensor_add(zt[:], ps_og[:], biasg_rep[:])
                      nc.scalar.activation(
                          zr_sb[:, g * OG : (g + 1) * OG], zt[:], AF.Sigmoid
                      )
                  # zs = z * state (single strided op)
                  nc.vector.tensor_mul(
                      zs_grp[:].rearrange("p (g o) -> p g o", o=DO),
                      zr_sb[:].rearrange("p (g o) -> p g o", o=OG)[:, :, 0:DO],
                      state_grp[:].rearrange("p (g o) -> p g o", o=DO),
                  )

              # zs -> dram rows [node | (b,o)]
              for jj in range(4):
                  nc.gpsimd.dma_start(
                      d_zs_in[:]
                      .rearrange("(g jj) (b o) -> jj b g o", jj=4, o=DO)[jj],
                      zs_grp[32 * jj : 32 * jj + 32, :]
                      .rearrange("b (g o) -> b g o", o=DO),
                  )
              nc.gpsimd.collective_compute(
                  "AllGather",
                  ALU.bypass,
                  replica_groups=[list(range(NCORES))],
                  ins=[d_zs_in.opt()],
                  outs=[d_zs_out.opt()],
              )

              # candT rows 64:128 = (z*state)^T for local nodes (PE transpose)
              with tc.tile_pool(name="psz", bufs=2, space="PSUM") as psz:
                  for g in range(NG):
                      ps_zt = psz.tile([DO, 128], F16, tag="ps_zt")
                      nc.tensor.transpose(
                          ps_zt[:], zs_grp[:, g * DO : (g + 1) * DO], ident16[:]
                      )
                      dst = (
                          candT[DI:C, :]
                          .rearrange("c (b n) -> c b n", n=NL)[
                              :, :, g * 4 : g * 4 + 4
                          ]
                      )
                      src = ps_zt[:].rearrange("c (jj b) -> c b jj", jj=4)
                      nc.vector.tensor_copy(dst, src)

              # xg2uT rows 0:64 = xg2T rows 0:64 (A @ x part, already scaled)
              nc.vector.tensor_copy(xg2uT[0:DI, :], xg2T[0:DI, :])

              if phases < 3:
                  raise _Stop()
              # ---------- upd PV (zs part only) ----------
              with tc.tile_pool(name="psu", bufs=2, space="PSUM") as psu:
                  for b in range(B):
                      zs_b = stg.tile([128, 8, DO], F16, tag="zs_b")
                      nc.gpsimd.dma_start(
                          zs_b[:],
                          d_zs_out[:]
                          .rearrange("q m (b o) -> m q b o", o=DO)[:, :, b, :],
                      )
                      exp_rb = stg.tile([128, 8 * NL], F16, tag="exp_b")
                      nc.gpsimd.dma_start(exp_rb[:], d_exp[b])
                      ps_xu = psu.tile([DO, NL], F32, tag="xu")
                      for q in range(8):
                          nc.tensor.matmul(
                              ps_xu[:], zs_b[:, q, :],
                              exp_rb[:, q * NL : (q + 1) * NL],
                              start=(q == 0), stop=(q == 7),
                          )
                      ps_rep = psu.tile([128, NL], F32, tag="rep_u")
                      nc.tensor.matmul(
                          ps_rep[:], ones16r[:],
                          sinv_sb[:, b * NL : (b + 1) * NL],
                          start=True, stop=True,
                      )
                      rep_sb = stg.tile([128, NL], F32, tag="rep_u_sb")
                      nc.vector.tensor_copy(rep_sb[:], ps_rep[:])
                      nc.vector.tensor_mul(
                          xg2uT[DI:C, b * NL : (b + 1) * NL],
                          ps_xu[:], rep_sb[0:DO, :],
                      )

              if phases < 4:
                  raise _Stop()
              # ---------- w-gen upd + upd out-mm ----------
              wgen(pu16, OU)

              wuview = (
                  wslab[:, : 2 * OU * NL]
                  .rearrange("c (k o n) -> c k o n", k=2, o=OU)
              )
              with tc.tile_pool(name="psou", bufs=3, space="PSUM") as psou:
                  for g in range(NG):
                      ps_ou = psou.tile([128, OU], F32, tag="ou")
                      for jj in range(4):
                          n_ = g * 4 + jj
                          for ki, src in ((0, candT), (1, xg2uT)):
                              lhs = src[:].rearrange("c (b n) -> c n b", n=NL)[:, n_, :]
                              rhs = wuview[:, ki, :, n_]
                              nc.tensor.matmul(
                                  ps_ou[32 * jj : 32 * jj + 32, :],
                                  lhs, rhs,
                                  start=(ki == 0), stop=(ki == 1),
                                  tile_position=(0, 32 * jj),
                              )
                      tt = stg.tile([128, OU], F32, tag="tt")
                      nc.vector.tensor_add(tt[:], ps_ou[:], biasu_rep[:])
                      nc.scalar.activation(
                          hc_sb[:, g * OU : (g + 1) * OU], tt[:], AF.Tanh
                      )

              # ---------- final combine: h = r*(state - hc) + hc ----------
              nc.vector.tensor_sub(t1_sb[:], state_grp[:], hc_sb[:])
              nc.vector.tensor_mul(
                  t1_sb[:].rearrange("p (g o) -> p g o", o=DO),
                  t1_sb[:].rearrange("p (g o) -> p g o", o=DO),
                  zr_sb[:].rearrange("p (g o) -> p g o", o=OG)[:, :, DO:OG],
              )
              nc.vector.tensor_add(h_sb[:], t1_sb[:], hc_sb[:])
          except _Stop:
              pass
          nc.gpsimd.dma_start(h_out[:], h_sb[:])

    nc.finalize()
    return nc


def _get_nc():
    phases = float(os.environ.get("KERNEL_PHASES", "4"))
    key = f"nc{phases}"
    if key not in _CACHE:
        _CACHE[key] = _build(phases)
    return _CACHE[key]


def kernel(x, state, node_emb, time_emb, gate_w, gate_b, gate_gamma, gate_beta,
           upd_w, upd_b, upd_gamma, upd_beta):
    global LAST_RESULT
    x = np.asarray(x, np.float32)
    state = np.asarray(state, np.float32)
    node_emb = np.asarray(node_emb, np.float32)
    time_emb = np.asarray(time_emb, np.float32)
    gate_w = np.asarray(gate_w, np.float32)
    gate_b = np.asarray(gate_b, np.float32)
    upd_w = np.asarray(upd_w, np.float32)
    upd_b = np.asarray(upd_b, np.float32)

    shared = (
        np.array_equal(np.asarray(gate_gamma), np.ones(D, np.float32))
        and np.array_equal(np.asarray(upd_gamma), np.ones(D, np.float32))
        and np.array_equal(np.asarray(gate_beta), np.zeros(D, np.float32))
        and np.array_equal(np.asarray(upd_beta), np.zeros(D, np.float32))
    )
    if not shared:
        return _np_reference(x, state, node_emb, time_emb, gate_w, gate_b,
                             gate_gamma, gate_beta, upd_w, upd_b, upd_gamma,
                             upd_beta)

    if os.environ.get("BASS_TRACE"):
        _install_prof_shim()

    from concourse.bass_utils import run_bass_kernel_spmd

    nc = _get_nc()

    x16 = x.astype(np.float16)
    st16 = state.astype(np.float16)
    xT16 = np.ascontiguousarray(x.transpose(2, 0, 1)).astype(np.float16)
    stT16 = np.ascontiguousarray(state.transpose(2, 0, 1)).astype(np.float16)
    neT16 = np.ascontiguousarray(node_emb.T).astype(np.float16)
    teT16 = np.ascontiguousarray(time_emb.T).astype(np.float16)
    pg16 = np.ascontiguousarray(gate_w.transpose(1, 3, 0, 2)).astype(np.float16)
    pu16 = np.ascontiguousarray(upd_w.transpose(1, 3, 0, 2)).astype(np.float16)

    in_maps = []
    for c in range(NCORES):
        nlo = c * NL
        in_maps.append({
            "ne_f32": np.ascontiguousarray(node_emb[nlo : nlo + NL]),
            "neT16": np.ascontiguousarray(neT16[:, nlo : nlo + NL]),
            "te_f32": time_emb,
            "teT16": teT16,
            "x16": x16,
            "st16": st16,
            "xT16": np.ascontiguousarray(xT16[:, :, nlo : nlo + NL]),
            "stT16": np.ascontiguousarray(stT16[:, :, nlo : nlo + NL]),
            "st_loc": np.ascontiguousarray(state[:, nlo : nlo + NL, :]),
            "pg16": pg16,
            "pu16": pu16,
            "gb16": gate_b.astype(np.float16),
            "ub16": upd_b.astype(np.float16),
        })

    res = run_bass_kernel_spmd(
        nc, in_maps, list(range(NCORES)),
        trace=bool(os.environ.get("BASS_TRACE")),
    )
    LAST_RESULT = res

    h = np.empty((B, N, DO), np.float32)
    for c in range(NCORES):
        ho = res.results[c]["h_out"].reshape(4, 32, NG, DO)  # [jj, b, g, o]
        h[:, c * NL : (c + 1) * NL, :] = (
            ho.transpose(1, 2, 0, 3).reshape(B, NL, DO)
        )
    return h



# revision 2
# speedup vs baseline: 1.0100x; 1.0100x over previous
"""Trainium2 Bass kernel v2 for the GRU-GCN cell (nn_GRUCell).

Sharding: 8 cores, node-parallel (NL=128 nodes/core, all B=32 batches).

Key structure vs v1:
- logits per batch computed [n, m] with two N=512 matmuls into one 2-bank
  PSUM tile; ONE big exp activation with accum_out giving softmax row-sums
  for free; DVE reciprocal + per-partition scale -> A already normalized.
- A^T chunks (the [m, n] layout needed by xg2/PV matmuls) produced by 8 PE
  transposes per batch into a single f16 PSUM bank, copied out with one
  big DVE copy; A^T stays SBUF-resident for the upd PV pass (no DRAM
  round-trip of the attention matrix).
- Activations stored n-major [C, (n, b)] so out-mm stationary slices are
  contiguous; weight slab stored [C, (k, g, jj, o)] so out-mm moving
  operands are contiguous.
- Out-mm col-tiled 4x (tile_position), bias added via one big
  stride-0-broadcast tensor_tensor per 4-group slab, one big sigmoid/tanh.
- wgen row-tiled 2x (two K=64 matmuls in the two 64-row halves of the PE).
- wgen-upd + local zs transposes overlap AllGather#2.
"""

import os
import sys

sys.path.insert(0, "/opt/trn_rl_repo")
import numpy as np

B, N, D = 32, 1024, 64
DI = DO = 64
C = DI + DO  # 128
OG, OU = 2 * DO, DO  # 128, 64
NCORES = 8
NL = N // NCORES  # 128
NG = NL // 4  # 32 groups of 4 nodes
EPS = 1e-12

USE_SHARED_CC = os.environ.get("K2_SHARED", "1") == "1"
ROWTILE_WGEN = os.environ.get("K2_ROWTILE", "1") == "1"
COLTILE_PV = os.environ.get("K2_COLPV", "1") == "1"

_CACHE = {}
LAST_RESULT = None


def _np_reference(x, state, node_emb, time_emb, gate_w, gate_b, gate_gamma,
                  gate_beta, upd_w, upd_b, upd_gamma, upd_beta):
    def _ln(v, g, b2):
        mu = v.mean(-1, keepdims=True)
        var = ((v - mu) ** 2).mean(-1, keepdims=True)
        return (v - mu) / np.sqrt(var + EPS) * g + b2

    def _gcn(xg, w_pool, b_pool, g, b2):
        emb = _ln(node_emb[None] + time_emb[:, None], g, b2)
        logits = np.einsum("bnd,bmd->bnm", emb, emb, optimize=True)
        a = np.exp(logits - logits.max(-1, keepdims=True))
        a /= a.sum(-1, keepdims=True)
        xg2 = np.einsum("bnm,bmc->bnc", a, xg, optimize=True)
        w = np.einsum("nd,dkio->nkio", node_emb, w_pool, optimize=True)
        bias = time_emb @ b_pool
        return (np.einsum("bni,nio->bno", xg, w[:, 0], optimize=True)
                + np.einsum("bni,nio->bno", xg2, w[:, 1], optimize=True)
                + bias[:, None, :])

    inp = np.concatenate([x, state], -1)
    zr = 1.0 / (1.0 + np.exp(-_gcn(inp, gate_w, gate_b, gate_gamma, gate_beta)))
    z, r = zr[..., :DO], zr[..., DO:]
    cand = np.concatenate([x, z * state], -1)
    hc = np.tanh(_gcn(cand, upd_w, upd_b, upd_gamma, upd_beta))
    return (r * state + (1.0 - r) * hc).astype(np.float32)


def _install_prof_shim():
    import types

    if "antenv.axon_hooks" in sys.modules:
        return
    try:
        from trn_agent_boot.trn_boot import _ntff_profile_via_ctypes

        hook = _ntff_profile_via_ctypes("/opt/axon/libaxon_pjrt.so")
    except Exception:
        hook = None
    mod = types.ModuleType("antenv.axon_hooks")
    mod.get_axon_ntff_profile_hook = lambda: hook

    def _set(h):
        mod.get_axon_ntff_profile_hook = lambda: h

    mod.set_axon_ntff_profile_hook = _set
    sys.modules["antenv.axon_hooks"] = mod
    try:
        import antenv

        antenv.axon_hooks = mod
    except Exception:
        pass


def _build():
    import concourse.bacc as bacc
    import concourse.bass as bass
    import concourse.mybir as mybir
    from concourse.tile import TileContext
    from concourse.masks import make_identity

    F16 = mybir.dt.float16
    F32 = mybir.dt.float32
    AF = mybir.ActivationFunctionType
    ALU = mybir.AluOpType
    X = mybir.AxisListType.X

    nc = bacc.Bacc()

    def pin(name, shape, dt=F16):
        return nc.declare_dram_parameter(name, shape, dt, isOutput=False)

    ne_f32 = pin("ne_f32", [NL, D], F32)     # local node_emb rows (LN input)
    neT16 = pin("neT16", [D, NL])            # node_embT local (wgen rhs)
    te_f32 = pin("te_f32", [1, B * D], F32)  # time_emb flat row (LN bcast)
    teT16 = pin("teT16", [D, B])             # bias matmul lhsT
    inp16 = pin("inp16", [B, N, C])          # host-concat [x, state]
    xTn16 = pin("xTn16", [DI, NL, B])        # n-major local transposed
    stTn16 = pin("stTn16", [DO, NL, B])
    st_loc = pin("st_loc", [B, NL, DO])      # f16 group-layout state
    pg16 = pin("pg16", [2, OG, D, C])        # gate pool permuted (k,o,d,i)
    pu16 = pin("pu16", [2, OU, D, C])
    gb16 = pin("gb16", [D, OG])
    ub16 = pin("ub16", [D, OU])
    h_out = nc.declare_dram_parameter("h_out", [128, NG * DO], F16, isOutput=True)

    AS = "Shared" if USE_SHARED_CC else "Local"

    BH = B // 2  # collective batch-half

    with TileContext(nc) as tc:
        # ---- DRAM scratch (collectives split into batch halves) ----
        BQ = B // 4  # AG#1 quarter
        d_e_in = []
        d_e_out = []
        for qi in range(4):
            ti, _ = tc.tile([D, BQ * NL], F16, space="DRAM", name=f"d_e_in{qi}")
            to, _ = tc.tile([NCORES, D, BQ * NL], F16, space="DRAM",
                            addr_space=AS, name=f"d_e_out{qi}")
            d_e_in.append(ti)
            d_e_out.append(to)
        d_z_inA, _ = tc.tile([NL, BH * DO], F16, space="DRAM", name="d_z_inA")
        d_z_inB, _ = tc.tile([NL, BH * DO], F16, space="DRAM", name="d_z_inB")
        d_z_outA, _ = tc.tile([NCORES, NL, BH * DO], F16, space="DRAM",
                              addr_space=AS, name="d_z_outA")
        d_z_outB, _ = tc.tile([NCORES, NL, BH * DO], F16, space="DRAM",
                              addr_space=AS, name="d_z_outB")
        d_z_in = (d_z_inA, d_z_inB)
        d_z_out = (d_z_outA, d_z_outB)

        with (
            tc.tile_pool(name="const", bufs=1) as cpool,
            tc.tile_pool(name="big", bufs=1) as big,
            tc.tile_pool(name="stg", bufs=2) as stg,
        ):
            # ---------- constants / persistent ----------
            ones_row32 = cpool.tile([1, 128], F32, tag="ones_row32")
            nc.gpsimd.memset(ones_row32[:], 1.0)
            ident16 = cpool.tile([128, 128], F16, tag="ident16")
            make_identity(nc, ident16[:])
            neg64_col = cpool.tile([128, 1], F32, tag="neg64_col")
            nc.gpsimd.memset(neg64_col[:], -64.0)

            ne_sb = cpool.tile([NL, D], F32, tag="ne_sb")
            nc.gpsimd.dma_start(ne_sb[:], ne_f32[:])
            te_row = cpool.tile([1, B * D], F32, tag="te_row")
            nc.gpsimd.dma_start(te_row[:], te_f32[:])
            neT2 = cpool.tile([128, NL], F16, tag="neT2")
            nc.gpsimd.dma_start(neT2[0:D, :], neT16[:])
            nc.gpsimd.dma_start(neT2[D:128, :], neT16[:])
            teT_sb = cpool.tile([D, B], F16, tag="teT_sb")
            nc.gpsimd.dma_start(teT_sb[:], teT16[:])
            gb_sb = cpool.tile([D, OG], F16, tag="gb_sb")
            nc.gpsimd.dma_start(gb_sb[:], gb16[:])
            ub_sb = cpool.tile([D, OU], F16, tag="ub_sb")
            nc.gpsimd.dma_start(ub_sb[:], ub16[:])

            # persistent big tiles
            embT_loc = big.tile([D, B * NL], F16, tag="embT_loc")
            inpT = big.tile([C, NL * B], F16, tag="inpT")     # n-major
            xg2T = big.tile([C, NL * B], F16, tag="xg2T")     # n-major
            A_T = big.tile([128, B * 8 * NL], F16, tag="A_T")  # 8.4MB
            wslab = big.tile([C, 2 * NG * 4 * OG], F16, tag="wslab")  # 8.4MB
            zr_sb = big.tile([128, NG * OG], F16, tag="zr_sb")
            state_grp = big.tile([128, NG * DO], F16, tag="state_grp")
            zs_grp = big.tile([128, NG * DO], F16, tag="zs_grp")
            hc_sb = big.tile([128, NG * DO], F16, tag="hc_sb")
            biasg_rep = big.tile([128, OG], F32, tag="biasg_rep")
            biasu_rep = big.tile([128, OU], F32, tag="biasu_rep")

            # n-major activations: rows 0:64 = x, 64:128 = state (later zs)
            nc.gpsimd.dma_start(
                inpT[0:DI, :], xTn16[:].rearrange("d n b -> d (n b)"))
            nc.gpsimd.dma_start(
                inpT[DI:C, :], stTn16[:].rearrange("d n b -> d (n b)"))
            # state group tiles [32*jj + b | g*64 + o]
            for jj in range(4):
                nc.gpsimd.dma_start(
                    state_grp[32 * jj: 32 * jj + 32, :]
                    .rearrange("b (g o) -> b g o", o=DO),
                    st_loc[:].rearrange("b (g jj) o -> b g jj o", jj=4)[:, :, jj, :],
                )

            # ---------- bias = time_emb @ pool_b, replicated ----------
            with tc.tile_pool(name="psb", bufs=1, space="PSUM") as psb:
                ps_bg = psb.tile([B, OG], F32, tag="ps_bg")
                nc.tensor.matmul(ps_bg[:], teT_sb[:], gb_sb[:], start=True, stop=True)
                bg_row = stg.tile([B, OG], F32, tag="bg_row")
                nc.vector.tensor_copy(bg_row[:], ps_bg[:])
                ps_bu = psb.tile([B, OU], F32, tag="ps_bu")
                nc.tensor.matmul(ps_bu[:], teT_sb[:], ub_sb[:], start=True, stop=True)
                bu_row = stg.tile([B, OU], F32, tag="bu_row")
                nc.vector.tensor_copy(bu_row[:], ps_bu[:])
                for jj in range(4):
                    nc.gpsimd.dma_start(biasg_rep[32 * jj: 32 * jj + 32, :], bg_row[:])
                    nc.gpsimd.dma_start(biasu_rep[32 * jj: 32 * jj + 32, :], bu_row[:])

            # ---------- phase E: layernorm + transpose ----------
            with (
                tc.tile_pool(name="embu", bufs=1) as embu,
                tc.tile_pool(name="pse", bufs=1, space="PSUM") as pse,
            ):
                ps_u = pse.tile([128, B * D], F32, tag="ps_u")  # 4 banks
                for j in range(4):
                    nc.tensor.matmul(
                        ps_u[:, j * 512: (j + 1) * 512],
                        ones_row32[:],
                        te_row[:, j * 512: (j + 1) * 512],
                        start=True, stop=True,
                    )
                u_all = embu.tile([NL, B * D], F32, tag="u_all")
                u3 = u_all[:].rearrange("p (b d) -> p b d", d=D)
                ne3 = ne_sb[:].unsqueeze(1)  # [128, 1, 64]
                a0, a1 = bass.broadcast_tensor_aps(
                    ps_u[:].rearrange("p (b d) -> p b d", d=D), ne3)
                nc.vector.tensor_tensor(u3, a0, a1, ALU.add)
                scr = embu.tile([NL, B * D], F32, tag="scr")  # u^2, then (u-mu)
                nc.vector.tensor_mul(scr[:], u_all[:], u_all[:])
                sm = stg.tile([NL, B], F32, tag="sm")
                nc.vector.reduce_sum(sm[:], u3, axis=X)
                sq = stg.tile([NL, B], F32, tag="sq")
                nc.vector.reduce_sum(
                    sq[:], scr[:].rearrange("p (b d) -> p b d", d=D), axis=X)
                mu = stg.tile([NL, B], F32, tag="mu")
                nc.vector.tensor_scalar_mul(mu[:], sm[:], 1.0 / D)
                musq = stg.tile([NL, B], F32, tag="musq")
                nc.vector.tensor_mul(musq[:], mu[:], mu[:])
                var = stg.tile([NL, B], F32, tag="var")
                nc.vector.tensor_scalar_mul(var[:], sq[:], 1.0 / D)
                nc.vector.tensor_sub(var[:], var[:], musq[:])
                nc.vector.tensor_scalar_add(var[:], var[:], EPS)
                sd = stg.tile([NL, B], F32, tag="sd")
                nc.scalar.sqrt(sd[:], var[:])
                rstd = stg.tile([NL, B], F32, tag="rstd")
                nc.vector.reciprocal(rstd[:], sd[:])
                # embn = (u - mu)*rstd, big strided ops (bcast over d)
                embn = embu.tile([NL, B * D], F16, tag="embn")
                e3 = embn[:].rearrange("p (b d) -> p b d", d=D)
                t3 = scr[:].rearrange("p (b d) -> p b d", d=D)
                b0, b1 = bass.broadcast_tensor_aps(u3, mu[:].unsqueeze(2))
                nc.vector.tensor_tensor(t3, b0, b1, ALU.subtract)
                c0, c1 = bass.broadcast_tensor_aps(t3, rstd[:].unsqueeze(2))
                nc.vector.tensor_tensor(e3, c0, c1, ALU.mult)
                # transposes: [NL, D] -> [D, NL] per b, 8 per f16 psum bank;
                # fire each AG#1 quarter as soon as its block is ready
                with tc.tile_pool(name="pst", bufs=2, space="PSUM") as pst:
                    for bb8 in range(4):
                        ps_tr = pst.tile([D, 8 * NL], F16, tag="ps_tr")
                        for i in range(8):
                            b = bb8 * 8 + i
                            nc.tensor.transpose(
                                ps_tr[:, i * NL: (i + 1) * NL],
                                embn[:, b * D: (b + 1) * D],
                                ident16[:],
                            )
                        nc.vector.tensor_copy(
                            embT_loc[:, bb8 * 8 * NL: (bb8 + 1) * 8 * NL],
                            ps_tr[:],
                        )
                        nc.gpsimd.dma_start(
                            d_e_in[bb8][:],
                            embT_loc[:, bb8 * BQ * NL: (bb8 + 1) * BQ * NL])
                        nc.gpsimd.collective_compute(
                            "AllGather", ALU.bypass,
                            replica_groups=[list(range(NCORES))],
                            ins=[d_e_in[bb8].opt()], outs=[d_e_out[bb8].opt()],
                        )

            # ---------- wgen chunk machinery ----------
            # wslab free layout: k*(NG*4*n_o) + g*(4*n_o) + jj*n_o + o
            def wgen_pw_dma(pool_dram, pwstg, k, o0):
                # stage 8 o-slices [D, 8*C] of the permuted pool
                pw = pwstg.tile([D, 8 * C], F16, tag="pw")
                nc.gpsimd.dma_start(
                    pw[:].rearrange("d (o i) -> d o i", i=C),
                    pool_dram[k, o0: o0 + 8].rearrange("o d i -> d o i"),
                )
                return pw

            def wgen_chunk(n_o, psw, pw, half, k, o0c, eng_i):
                # 4 o's -> one psum bank -> one scatter-cast into wslab
                kstr = NG * 4 * n_o
                ps_w4 = psw.tile([C, 4 * NL], F32, tag="ps_w4")
                for j in range(4):
                    nc.tensor.matmul(
                        ps_w4[:, j * NL: (j + 1) * NL],
                        pw[:, (half * 4 + j) * C: (half * 4 + j + 1) * C],
                        neT2[0:D, :],
                        start=True, stop=True,
                    )
                src = ps_w4[:].rearrange("c (j g jj) -> c j g jj", j=4, jj=4)
                dst = (
                    wslab[:, k * kstr: (k + 1) * kstr]
                    .rearrange("c (g jj o) -> c g jj o", jj=4, o=n_o)
                    [:, :, :, o0c: o0c + 4]
                    .rearrange("c g jj o -> c o g jj")
                )
                if eng_i % 2 == 0:
                    nc.vector.tensor_copy(dst, src)
                else:
                    nc.scalar.activation(dst, src, AF.Copy)

            # ---------- wgen gate: runs during barrier + AG#1 window ------
            # deep PSUM ring so the MMs and scatter-casts pipeline freely
            with (
                tc.tile_pool(name="psw", bufs=6, space="PSUM") as psw,
                tc.tile_pool(name="pwstg", bufs=2) as pwstg,
            ):
                for blk in range(2 * OG // 8):
                    kw, ow = divmod(blk, OG // 8)
                    pw = wgen_pw_dma(pg16, pwstg, kw, ow * 8)
                    wgen_chunk(OG, psw, pw, 0, kw, ow * 8, blk)
                    wgen_chunk(OG, psw, pw, 1, kw, ow * 8 + 4, blk + 1)

            # ---------- gate loop (software-pipelined) ----------
            with (
                tc.tile_pool(name="psl", bufs=2, space="PSUM") as psl,
                tc.tile_pool(name="psa", bufs=2, space="PSUM") as psa,
                tc.tile_pool(name="psx", bufs=2, space="PSUM") as psx,
                tc.tile_pool(name="gstg", bufs=3) as gstg,
                tc.tile_pool(name="gstg2", bufs=3) as gstg2,
                tc.tile_pool(name="ascp", bufs=3) as ascp,
            ):
                it_t = {}
                asc_t = {}

                def stage_logits(b):
                    embT_b = gstg.tile([D, N], F16, tag="embT_b")
                    bh, bl = divmod(b, BQ)
                    nc.sync.dma_start(
                        embT_b[:].rearrange("d (q n) -> d q n", n=NL),
                        d_e_out[bh][:, :, bl * NL: (bl + 1) * NL]
                        .rearrange("q d n -> d q n"),
                    )
                    it_b = gstg2.tile([128, 8, C], F16, tag="it_b")
                    nc.gpsimd.dma_start(
                        it_b[:],
                        inp16[b].rearrange("(q m) d -> m q d", m=128),
                    )
                    it_t[b] = it_b
                    ps_l = psl.tile([NL, N], F32, tag="ps_l")  # 2 banks
                    lhs = embT_loc[:, b * NL: (b + 1) * NL]
                    nc.tensor.matmul(ps_l[:, 0:512], lhs, embT_b[:, 0:512],
                                     start=True, stop=True)
                    nc.tensor.matmul(ps_l[:, 512:1024], lhs, embT_b[:, 512:1024],
                                     start=True, stop=True)
                    # exp with accumulated row sums
                    asc = ascp.tile([NL, N], F16, tag="asc")
                    s_col = gstg.tile([NL, 1], F32, tag="s_col")
                    nc.scalar.activation(asc[:], ps_l[:], AF.Exp,
                                         bias=neg64_col[:], accum_out=s_col[:])
                    rinv = gstg.tile([NL, 1], F32, tag="rinv")
                    nc.vector.reciprocal(rinv[:], s_col[:])
                    nc.vector.tensor_scalar_mul(asc[:], asc[:], rinv[:])
                    asc_t[b] = asc

                def stage_transpose(b):
                    asc = asc_t.pop(b)
                    ps_at = psa.tile([128, N], F16, tag="ps_at")  # 1 bank
                    for q in range(8):
                        nc.tensor.transpose(
                            ps_at[:, q * NL: (q + 1) * NL],
                            asc[:, q * NL: (q + 1) * NL],
                            ident16[:],
                        )
                    nc.vector.tensor_copy(
                        A_T[:, b * 8 * NL: (b + 1) * 8 * NL], ps_at[:])

                def stage_xg2(b):
                    it_b = it_t.pop(b)
                    ps_x2 = psx.tile([C, NL], F32, tag="ps_x2")
                    for q in range(8):
                        nc.tensor.matmul(
                            ps_x2[:],
                            it_b[:, q, :],
                            A_T[:, (b * 8 + q) * NL: (b * 8 + q + 1) * NL],
                            start=(q == 0), stop=(q == 7),
                        )
                    dst = (xg2T[:].rearrange("c (n b) -> c n b", b=B)[:, :, b])
                    nc.vector.tensor_copy(dst, ps_x2[:])

                for bb in range(B + 2):
                    if bb < B:
                        stage_logits(bb)
                    if 1 <= bb < B + 1:
                        stage_transpose(bb - 1)
                    if 2 <= bb:
                        stage_xg2(bb - 2)

            # ---------- gate out-mm ----------
            def out_mm(n_o, src_k, wk_stride, act_fn, bias_rep, dest, gper):
                # gper groups per 512-col psum slab
                with (
                    tc.tile_pool(name="pso", bufs=2, space="PSUM") as pso,
                    tc.tile_pool(name="ostg", bufs=2) as ostg,
                ):
                    for gg in range(NG // gper):
                        ps_o = pso.tile([128, gper * n_o], F32, tag="ps_o")
                        for g4 in range(gper):
                            g = gg * gper + g4
                            for jj in range(4):
                                n_ = g * 4 + jj
                                for ki, src in src_k:
                                    rhs = wslab[:, ki * wk_stride + g * (4 * n_o)
                                                + jj * n_o: ki * wk_stride
                                                + g * (4 * n_o) + (jj + 1) * n_o]
                                    nc.tensor.matmul(
                                        ps_o[32 * jj: 32 * jj + 32,
                                             g4 * n_o: (g4 + 1) * n_o],
                                        src[:, n_ * B: (n_ + 1) * B],
                                        rhs,
                                        start=(ki == 0), stop=(ki == 1),
                                        tile_position=(0, 32 * jj),
                                    )
                        zt = ostg.tile([128, gper * n_o], F32, tag="zt")
                        z3 = zt[:].rearrange("p (g o) -> p g o", o=n_o)
                        a0, a1 = bass.broadcast_tensor_aps(
                            ps_o[:].rearrange("p (g o) -> p g o", o=n_o),
                            bias_rep[:].unsqueeze(1),
                        )
                        nc.vector.tensor_tensor(z3, a0, a1, ALU.add)
                        nc.scalar.activation(
                            dest[:, gg * gper * n_o: (gg + 1) * gper * n_o],
                            zt[:], act_fn)

            out_mm(OG, ((0, inpT), (1, xg2T)), NG * 4 * OG, AF.Sigmoid,
                   biasg_rep, zr_sb, 4)

            # zs = z * state (one strided op)
            nc.vector.tensor_mul(
                zs_grp[:].rearrange("p (g o) -> p g o", o=DO),
                zr_sb[:].rearrange("p (g o) -> p g o", o=OG)[:, :, 0:DO],
                state_grp[:].rearrange("p (g o) -> p g o", o=DO),
            )

            # zs -> dram rows [node | (b,o)] and AG#2 in two batch halves
            for h in range(2):
                for jj in range(4):
                    nc.gpsimd.dma_start(
                        d_z_in[h][:]
                        .rearrange("(g jj) (b o) -> jj b g o", jj=4, o=DO)[jj],
                        zs_grp[32 * jj + BH * h: 32 * jj + BH * h + BH, :]
                        .rearrange("b (g o) -> b g o", o=DO),
                    )
                nc.gpsimd.collective_compute(
                    "AllGather", ALU.bypass,
                    replica_groups=[list(range(NCORES))],
                    ins=[d_z_in[h].opt()], outs=[d_z_out[h].opt()],
                )

            # ---------- overlap AG#2: wgen upd + local zs transposes ----------
            with (
                tc.tile_pool(name="pswu", bufs=4, space="PSUM") as pswu,
                tc.tile_pool(name="pwstgu", bufs=2) as pwstgu,
            ):
                for blk in range(2 * OU // 8):
                    ku, ou = divmod(blk, OU // 8)
                    pw = wgen_pw_dma(pu16, pwstgu, ku, ou * 8)
                    wgen_chunk(OU, pswu, pw, 0, ku, ou * 8, blk)
                    wgen_chunk(OU, pswu, pw, 1, ku, ou * 8 + 4, blk + 1)

            # inpT rows 64:128 := (z*state)^T n-major (gate out-mm done)
            with tc.tile_pool(name="psz", bufs=2, space="PSUM") as psz:
                for g8 in range(4):
                    ps_zt = psz.tile([DO, 8 * 128], F16, tag="ps_zt")
                    for i in range(8):
                        g = g8 * 8 + i
                        nc.tensor.transpose(
                            ps_zt[:, i * 128: (i + 1) * 128],
                            zs_grp[:, g * DO: (g + 1) * DO],
                            ident16[:],
                        )
                    nc.vector.tensor_copy(
                        inpT[DI:C, g8 * 8 * 128: (g8 + 1) * 8 * 128],
                        ps_zt[:],
                    )

            # ---------- PV upd: xg2T rows 64:128 := (A @ zs)^T ----------
            with (
                tc.tile_pool(name="psp", bufs=2, space="PSUM") as psp,
                tc.tile_pool(name="pvstg", bufs=3) as pvstg,
            ):
                npair = B // 2 if COLTILE_PV else B
                for p in range(npair):
                    if COLTILE_PV:
                        bs = (2 * p, 2 * p + 1)
                    else:
                        bs = (p,)
                    zsb = pvstg.tile([128, 8, len(bs), DO], F16, tag="zsb")
                    bh, bl = divmod(bs[0], BH)
                    nc.sync.dma_start(
                        zsb[:],
                        d_z_out[bh][:]
                        .rearrange("q m (b o) -> m q b o", o=DO)
                        [:, :, bl: bl + len(bs), :],
                    )
                    ps_pv = psp.tile([128, NL], F32, tag="ps_pv")
                    for q in range(8):
                        for i, b in enumerate(bs):
                            nc.tensor.matmul(
                                ps_pv[64 * i: 64 * i + 64, :],
                                zsb[:, q, i, :],
                                A_T[:, (b * 8 + q) * NL: (b * 8 + q + 1) * NL],
                                start=(q == 0), stop=(q == 7),
                                tile_position=(0, 64 * i) if COLTILE_PV else None,
                            )
                    for i, b in enumerate(bs):
                        dst = (xg2T[DI:C, :]
                               .rearrange("c (n b) -> c n b", b=B)[:, :, b])
                        nc.vector.tensor_copy(dst, ps_pv[64 * i: 64 * i + 64, :])

            # ---------- upd out-mm ----------
            out_mm(OU, ((0, inpT), (1, xg2T)), NG * 4 * OU, AF.Tanh,
                   biasu_rep, hc_sb, 8)

            # ---------- combine: h = r*(state - hc) + hc (zs_grp as scratch) ----------
            nc.vector.tensor_sub(zs_grp[:], state_grp[:], hc_sb[:])
            nc.vector.tensor_mul(
                zs_grp[:].rearrange("p (g o) -> p g o", o=DO),
                zs_grp[:].rearrange("p (g o) -> p g o", o=DO),
                zr_sb[:].rearrange("p (g o) -> p g o", o=OG)[:, :, DO:OG],
            )
            nc.vector.tensor_add(zs_grp[:], zs_grp[:], hc_sb[:])
            nc.gpsimd.dma_start(h_out[:], zs_grp[:])

    nc.finalize()
    return nc


def _get_nc():
    if "nc" not in _CACHE:
        _CACHE["nc"] = _build()
    return _CACHE["nc"]


def kernel(x, state, node_emb, time_emb, gate_w, gate_b, gate_gamma, gate_beta,
           upd_w, upd_b, upd_gamma, upd_beta):
    global LAST_RESULT
    x = np.asarray(x, np.float32)
    state = np.asarray(state, np.float32)
    node_emb = np.asarray(node_emb, np.float32)
    time_emb = np.asarray(time_emb, np.float32)
    gate_w = np.asarray(gate_w, np.float32)
    gate_b = np.asarray(gate_b, np.float32)
    upd_w = np.asarray(upd_w, np.float32)
    upd_b = np.asarray(upd_b, np.float32)

    shared = (
        np.array_equal(np.asarray(gate_gamma), np.ones(D, np.float32))
        and np.array_equal(np.asarray(upd_gamma), np.ones(D, np.float32))
        and np.array_equal(np.asarray(gate_beta), np.zeros(D, np.float32))
        and np.array_equal(np.asarray(upd_beta), np.zeros(D, np.float32))
    )
    if not shared:
        return _np_reference(x, state, node_emb, time_emb, gate_w, gate_b,
                             gate_gamma, gate_beta, upd_w, upd_b, upd_gamma,
                             upd_beta)

    if os.environ.get("BASS_TRACE"):
        _install_prof_shim()

    from concourse.bass_utils import run_bass_kernel_spmd

    nc = _get_nc()

    inp16 = np.concatenate([x, state], axis=-1).astype(np.float16)  # [B,N,C]
    xTn = np.ascontiguousarray(x.transpose(2, 1, 0)).astype(np.float16)   # [DI,N,B]
    stTn = np.ascontiguousarray(state.transpose(2, 1, 0)).astype(np.float16)
    neT16 = np.ascontiguousarray(node_emb.T).astype(np.float16)
    teT16 = np.ascontiguousarray(time_emb.T).astype(np.float16)
    pg16 = np.ascontiguousarray(gate_w.transpose(1, 3, 0, 2)).astype(np.float16)
    pu16 = np.ascontiguousarray(upd_w.transpose(1, 3, 0, 2)).astype(np.float16)
    te_row = np.ascontiguousarray(time_emb.reshape(1, B * D))

    in_maps = []
    for c in range(NCORES):
        nlo = c * NL
        in_maps.append({
            "ne_f32": np.ascontiguousarray(node_emb[nlo: nlo + NL]),
            "neT16": np.ascontiguousarray(neT16[:, nlo: nlo + NL]),
            "te_f32": te_row,
            "teT16": teT16,
            "inp16": inp16,
            "xTn16": np.ascontiguousarray(xTn[:, nlo: nlo + NL, :]),
            "stTn16": np.ascontiguousarray(stTn[:, nlo: nlo + NL, :]),
            "st_loc": np.ascontiguousarray(
                state[:, nlo: nlo + NL, :]).astype(np.float16),
            "pg16": pg16,
            "pu16": pu16,
            "gb16": gate_b.astype(np.float16),
            "ub16": upd_b.astype(np.float16),
        })

    res = run_bass_kernel_spmd(
        nc, in_maps, list(range(NCORES)),
        trace=bool(os.environ.get("BASS_TRACE")),
    )
    LAST_RESULT = res

    h = np.empty((B, N, DO), np.float32)
    for c in range(NCORES):
        ho = res.results[c]["h_out"].astype(np.float32).reshape(4, 32, NG, DO)
        h[:, c * NL: (c + 1) * NL, :] = (
            ho.transpose(1, 2, 0, 3).reshape(B, NL, DO)
        )
    return h


# revision 3
# speedup vs baseline: 1.0279x; 1.0176x over previous
"""Trainium2 Bass kernel v2 for the GRU-GCN cell (nn_GRUCell).

Sharding: 8 cores, node-parallel (NL=128 nodes/core, all B=32 batches).

Key structure vs v1:
- logits per batch computed [n, m] with two N=512 matmuls into one 2-bank
  PSUM tile; ONE big exp activation with accum_out giving softmax row-sums
  for free; DVE reciprocal + per-partition scale -> A already normalized.
- A^T chunks (the [m, n] layout needed by xg2/PV matmuls) produced by 8 PE
  transposes per batch into a single f16 PSUM bank, copied out with one
  big DVE copy; A^T stays SBUF-resident for the upd PV pass (no DRAM
  round-trip of the attention matrix).
- Activations stored n-major [C, (n, b)] so out-mm stationary slices are
  contiguous; weight slab stored [C, (k, g, jj, o)] so out-mm moving
  operands are contiguous.
- Out-mm col-tiled 4x (tile_position), bias added via one big
  stride-0-broadcast tensor_tensor per 4-group slab, one big sigmoid/tanh.
- wgen row-tiled 2x (two K=64 matmuls in the two 64-row halves of the PE).
- wgen-upd + local zs transposes overlap AllGather#2.
"""

import os
import sys

sys.path.insert(0, "/opt/trn_rl_repo")
import numpy as np

B, N, D = 32, 1024, 64
DI = DO = 64
C = DI + DO  # 128
OG, OU = 2 * DO, DO  # 128, 64
NCORES = 8
NL = N // NCORES  # 128
NG = NL // 4  # 32 groups of 4 nodes
EPS = 1e-12

USE_SHARED_CC = os.environ.get("K2_SHARED", "1") == "1"
ROWTILE_WGEN = os.environ.get("K2_ROWTILE", "1") == "1"
COLTILE_PV = os.environ.get("K2_COLPV", "1") == "1"

_CACHE = {}
LAST_RESULT = None


def _np_reference(x, state, node_emb, time_emb, gate_w, gate_b, gate_gamma,
                  gate_beta, upd_w, upd_b, upd_gamma, upd_beta):
    def _ln(v, g, b2):
        mu = v.mean(-1, keepdims=True)
        var = ((v - mu) ** 2).mean(-1, keepdims=True)
        return (v - mu) / np.sqrt(var + EPS) * g + b2

    def _gcn(xg, w_pool, b_pool, g, b2):
        emb = _ln(node_emb[None] + time_emb[:, None], g, b2)
        logits = np.einsum("bnd,bmd->bnm", emb, emb, optimize=True)
        a = np.exp(logits - logits.max(-1, keepdims=True))
        a /= a.sum(-1, keepdims=True)
        xg2 = np.einsum("bnm,bmc->bnc", a, xg, optimize=True)
        w = np.einsum("nd,dkio->nkio", node_emb, w_pool, optimize=True)
        bias = time_emb @ b_pool
        return (np.einsum("bni,nio->bno", xg, w[:, 0], optimize=True)
                + np.einsum("bni,nio->bno", xg2, w[:, 1], optimize=True)
                + bias[:, None, :])

    inp = np.concatenate([x, state], -1)
    zr = 1.0 / (1.0 + np.exp(-_gcn(inp, gate_w, gate_b, gate_gamma, gate_beta)))
    z, r = zr[..., :DO], zr[..., DO:]
    cand = np.concatenate([x, z * state], -1)
    hc = np.tanh(_gcn(cand, upd_w, upd_b, upd_gamma, upd_beta))
    return (r * state + (1.0 - r) * hc).astype(np.float32)


def _install_prof_shim():
    import types

    if "antenv.axon_hooks" in sys.modules:
        return
    try:
        from trn_agent_boot.trn_boot import _ntff_profile_via_ctypes

        hook = _ntff_profile_via_ctypes("/opt/axon/libaxon_pjrt.so")
    except Exception:
        hook = None
    mod = types.ModuleType("antenv.axon_hooks")
    mod.get_axon_ntff_profile_hook = lambda: hook

    def _set(h):
        mod.get_axon_ntff_profile_hook = lambda: h

    mod.set_axon_ntff_profile_hook = _set
    sys.modules["antenv.axon_hooks"] = mod
    try:
        import antenv

        antenv.axon_hooks = mod
    except Exception:
        pass


def _build():
    import concourse.bacc as bacc
    import concourse.bass as bass
    import concourse.mybir as mybir
    from concourse.tile import TileContext
    from concourse.masks import make_identity

    F16 = mybir.dt.float16
    F32 = mybir.dt.float32
    AF = mybir.ActivationFunctionType
    ALU = mybir.AluOpType
    X = mybir.AxisListType.X

    nc = bacc.Bacc()

    def pin(name, shape, dt=F16):
        return nc.declare_dram_parameter(name, shape, dt, isOutput=False)

    ne_f32 = pin("ne_f32", [NL, D], F32)     # local node_emb rows (LN input)
    neT16 = pin("neT16", [D, NL])            # node_embT local (wgen rhs)
    te_f32 = pin("te_f32", [1, B * D], F32)  # time_emb flat row (LN bcast)
    teT16 = pin("teT16", [D, B])             # bias matmul lhsT
    inp16 = pin("inp16", [B, N, C])          # host-concat [x, state]
    xTn16 = pin("xTn16", [DI, NL, B])        # n-major local transposed
    stTn16 = pin("stTn16", [DO, NL, B])
    st_loc = pin("st_loc", [B, NL, DO])      # f16 group-layout state
    pg16 = pin("pg16", [2, OG, D, C])        # gate pool permuted (k,o,d,i)
    pu16 = pin("pu16", [2, OU, D, C])
    gb16 = pin("gb16", [D, OG])
    ub16 = pin("ub16", [D, OU])
    h_out = nc.declare_dram_parameter("h_out", [128, NG * DO], F16, isOutput=True)

    AS = "Shared" if USE_SHARED_CC else "Local"

    BH = B // 2  # collective batch-half

    with TileContext(nc) as tc:
        # ---- DRAM scratch (collectives split into batch halves) ----
        BQ = B // 4  # AG#1 quarter
        d_e_in = []
        d_e_out = []
        for qi in range(4):
            ti, _ = tc.tile([D, BQ * NL], F16, space="DRAM", name=f"d_e_in{qi}")
            to, _ = tc.tile([NCORES, D, BQ * NL], F16, space="DRAM",
                            addr_space=AS, name=f"d_e_out{qi}")
            d_e_in.append(ti)
            d_e_out.append(to)
        d_z_inA, _ = tc.tile([NL, BH * DO], F16, space="DRAM", name="d_z_inA")
        d_z_inB, _ = tc.tile([NL, BH * DO], F16, space="DRAM", name="d_z_inB")
        d_z_outA, _ = tc.tile([NCORES, NL, BH * DO], F16, space="DRAM",
                              addr_space=AS, name="d_z_outA")
        d_z_outB, _ = tc.tile([NCORES, NL, BH * DO], F16, space="DRAM",
                              addr_space=AS, name="d_z_outB")
        d_z_in = (d_z_inA, d_z_inB)
        d_z_out = (d_z_outA, d_z_outB)

        with (
            tc.tile_pool(name="const", bufs=1) as cpool,
            tc.tile_pool(name="big", bufs=1) as big,
            tc.tile_pool(name="stg", bufs=2) as stg,
        ):
            # ---------- constants / persistent ----------
            ones_row32 = cpool.tile([1, 128], F32, tag="ones_row32")
            nc.gpsimd.memset(ones_row32[:], 1.0)
            ident16 = cpool.tile([128, 128], F16, tag="ident16")
            make_identity(nc, ident16[:])
            neg64_col = cpool.tile([128, 1], F32, tag="neg64_col")
            nc.gpsimd.memset(neg64_col[:], -64.0)

            ne_sb = cpool.tile([NL, D], F32, tag="ne_sb")
            nc.gpsimd.dma_start(ne_sb[:], ne_f32[:])
            te_row = cpool.tile([1, B * D], F32, tag="te_row")
            nc.gpsimd.dma_start(te_row[:], te_f32[:])
            neT2 = cpool.tile([128, NL], F16, tag="neT2")
            nc.gpsimd.dma_start(neT2[0:D, :], neT16[:])
            nc.gpsimd.dma_start(neT2[D:128, :], neT16[:])
            teT_sb = cpool.tile([D, B], F16, tag="teT_sb")
            nc.gpsimd.dma_start(teT_sb[:], teT16[:])
            gb_sb = cpool.tile([D, OG], F16, tag="gb_sb")
            nc.gpsimd.dma_start(gb_sb[:], gb16[:])
            ub_sb = cpool.tile([D, OU], F16, tag="ub_sb")
            nc.gpsimd.dma_start(ub_sb[:], ub16[:])

            # persistent big tiles
            embT_loc = big.tile([D, B * NL], F16, tag="embT_loc")
            inpT = big.tile([C, NL * B], F16, tag="inpT")     # n-major
            xg2T = big.tile([C, NL * B], F16, tag="xg2T")     # n-major
            A_T = big.tile([128, B * 8 * NL], F16, tag="A_T")  # 8.4MB
            wslab = big.tile([C, 2 * NG * 4 * OG], F16, tag="wslab")  # 8.4MB
            zr_sb = big.tile([128, NG * OG], F16, tag="zr_sb")
            state_grp = big.tile([128, NG * DO], F16, tag="state_grp")
            zs_grp = big.tile([128, NG * DO], F16, tag="zs_grp")
            hc_sb = big.tile([128, NG * DO], F16, tag="hc_sb")
            biasg_rep = big.tile([128, OG], F32, tag="biasg_rep")
            biasu_rep = big.tile([128, OU], F32, tag="biasu_rep")

            # n-major activations: rows 0:64 = x, 64:128 = state (later zs)
            nc.gpsimd.dma_start(
                inpT[0:DI, :], xTn16[:].rearrange("d n b -> d (n b)"))
            nc.gpsimd.dma_start(
                inpT[DI:C, :], stTn16[:].rearrange("d n b -> d (n b)"))
            # state group tiles [32*jj + b | g*64 + o]
            for jj in range(4):
                nc.gpsimd.dma_start(
                    state_grp[32 * jj: 32 * jj + 32, :]
                    .rearrange("b (g o) -> b g o", o=DO),
                    st_loc[:].rearrange("b (g jj) o -> b g jj o", jj=4)[:, :, jj, :],
                )

            # ---------- bias = time_emb @ pool_b, replicated ----------
            with tc.tile_pool(name="psb", bufs=1, space="PSUM") as psb:
                ps_bg = psb.tile([B, OG], F32, tag="ps_bg")
                nc.tensor.matmul(ps_bg[:], teT_sb[:], gb_sb[:], start=True, stop=True)
                bg_row = stg.tile([B, OG], F32, tag="bg_row")
                nc.vector.tensor_copy(bg_row[:], ps_bg[:])
                ps_bu = psb.tile([B, OU], F32, tag="ps_bu")
                nc.tensor.matmul(ps_bu[:], teT_sb[:], ub_sb[:], start=True, stop=True)
                bu_row = stg.tile([B, OU], F32, tag="bu_row")
                nc.vector.tensor_copy(bu_row[:], ps_bu[:])
                for jj in range(4):
                    nc.gpsimd.dma_start(biasg_rep[32 * jj: 32 * jj + 32, :], bg_row[:])
                    nc.gpsimd.dma_start(biasu_rep[32 * jj: 32 * jj + 32, :], bu_row[:])

            # ---------- phase E: layernorm + transpose ----------
            with (
                tc.tile_pool(name="embu", bufs=1) as embu,
                tc.tile_pool(name="pse", bufs=1, space="PSUM") as pse,
            ):
                ps_u = pse.tile([128, B * D], F32, tag="ps_u")  # 4 banks
                for j in range(4):
                    nc.tensor.matmul(
                        ps_u[:, j * 512: (j + 1) * 512],
                        ones_row32[:],
                        te_row[:, j * 512: (j + 1) * 512],
                        start=True, stop=True,
                    )
                u_all = embu.tile([NL, B * D], F32, tag="u_all")
                u3 = u_all[:].rearrange("p (b d) -> p b d", d=D)
                ne3 = ne_sb[:].unsqueeze(1)  # [128, 1, 64]
                a0, a1 = bass.broadcast_tensor_aps(
                    ps_u[:].rearrange("p (b d) -> p b d", d=D), ne3)
                nc.vector.tensor_tensor(u3, a0, a1, ALU.add)
                scr = embu.tile([NL, B * D], F32, tag="scr")  # u^2, then (u-mu)
                nc.vector.tensor_mul(scr[:], u_all[:], u_all[:])
                sm = stg.tile([NL, B], F32, tag="sm")
                nc.vector.reduce_sum(sm[:], u3, axis=X)
                sq = stg.tile([NL, B], F32, tag="sq")
                nc.vector.reduce_sum(
                    sq[:], scr[:].rearrange("p (b d) -> p b d", d=D), axis=X)
                mu = stg.tile([NL, B], F32, tag="mu")
                nc.vector.tensor_scalar_mul(mu[:], sm[:], 1.0 / D)
                musq = stg.tile([NL, B], F32, tag="musq")
                nc.vector.tensor_mul(musq[:], mu[:], mu[:])
                var = stg.tile([NL, B], F32, tag="var")
                nc.vector.tensor_scalar_mul(var[:], sq[:], 1.0 / D)
                nc.vector.tensor_sub(var[:], var[:], musq[:])
                nc.vector.tensor_scalar_add(var[:], var[:], EPS)
                sd = stg.tile([NL, B], F32, tag="sd")
                nc.scalar.sqrt(sd[:], var[:])
                rstd = stg.tile([NL, B], F32, tag="rstd")
                nc.vector.reciprocal(rstd[:], sd[:])
                # embn = (u - mu)*rstd, big strided ops (bcast over d)
                embn = embu.tile([NL, B * D], F16, tag="embn")
                e3 = embn[:].rearrange("p (b d) -> p b d", d=D)
                t3 = scr[:].rearrange("p (b d) -> p b d", d=D)
                b0, b1 = bass.broadcast_tensor_aps(u3, mu[:].unsqueeze(2))
                nc.vector.tensor_tensor(t3, b0, b1, ALU.subtract)
                c0, c1 = bass.broadcast_tensor_aps(t3, rstd[:].unsqueeze(2))
                nc.vector.tensor_tensor(e3, c0, c1, ALU.mult)
                # transposes: [NL, D] -> [D, NL] per b, 8 per f16 psum bank;
                # fire each AG#1 quarter as soon as its block is ready
                with tc.tile_pool(name="pst", bufs=2, space="PSUM") as pst:
                    for bb8 in range(4):
                        ps_tr = pst.tile([D, 8 * NL], F16, tag="ps_tr")
                        for i in range(8):
                            b = bb8 * 8 + i
                            nc.tensor.transpose(
                                ps_tr[:, i * NL: (i + 1) * NL],
                                embn[:, b * D: (b + 1) * D],
                                ident16[:],
                            )
                        nc.vector.tensor_copy(
                            embT_loc[:, bb8 * 8 * NL: (bb8 + 1) * 8 * NL],
                            ps_tr[:],
                        )
                        nc.gpsimd.dma_start(
                            d_e_in[bb8][:],
                            embT_loc[:, bb8 * BQ * NL: (bb8 + 1) * BQ * NL])
                        nc.gpsimd.collective_compute(
                            "AllGather", ALU.bypass,
                            replica_groups=[list(range(NCORES))],
                            ins=[d_e_in[bb8].opt()], outs=[d_e_out[bb8].opt()],
                        )

            # ---------- wgen chunk machinery ----------
            # wslab free layout: k*(NG*4*n_o) + g*(4*n_o) + jj*n_o + o
            def wgen_pw_dma(pool_dram, pwstg, k, o0):
                # stage 8 o-slices [D, 8*C] of the permuted pool
                pw = pwstg.tile([D, 8 * C], F16, tag="pw")
                nc.gpsimd.dma_start(
                    pw[:].rearrange("d (o i) -> d o i", i=C),
                    pool_dram[k, o0: o0 + 8].rearrange("o d i -> d o i"),
                )
                return pw

            def wgen_chunk(n_o, psw, wstg, pw, half, k, o0c, eng_i):
                # 4 o's -> one psum bank -> contiguous cast (DVE/ACT) ->
                # f16 SBUF->SBUF scatter on the otherwise-idle gpsimd engine
                kstr = NG * 4 * n_o
                ps_w4 = psw.tile([C, 4 * NL], F32, tag="ps_w4")
                for j in range(4):
                    nc.tensor.matmul(
                        ps_w4[:, j * NL: (j + 1) * NL],
                        pw[:, (half * 4 + j) * C: (half * 4 + j + 1) * C],
                        neT2[0:D, :],
                        start=True, stop=True,
                    )
                # wslab inner layout (g, o, jj): scatter writes 16-elem
                # (32B) contiguous runs per g
                src = ps_w4[:].rearrange("c (j g jj) -> c g j jj", j=4, jj=4)
                dst = (
                    wslab[:, k * kstr: (k + 1) * kstr]
                    .rearrange("c (g o jj) -> c g o jj", o=n_o, jj=4)
                    [:, :, o0c: o0c + 4, :]
                )
                if eng_i % 2 == 0:
                    nc.vector.tensor_copy(dst, src)
                else:
                    nc.scalar.activation(dst, src, AF.Copy)

            # ---------- wgen gate: runs during barrier + AG#1 window ------
            # deep PSUM ring so the MMs and copies pipeline freely
            with (
                tc.tile_pool(name="psw", bufs=6, space="PSUM") as psw,
                tc.tile_pool(name="pwstg", bufs=2) as pwstg,
            ):
                wstg = None
                for blk in range(2 * OG // 8):
                    kw, ow = divmod(blk, OG // 8)
                    pw = wgen_pw_dma(pg16, pwstg, kw, ow * 8)
                    wgen_chunk(OG, psw, wstg, pw, 0, kw, ow * 8, blk)
                    wgen_chunk(OG, psw, wstg, pw, 1, kw, ow * 8 + 4, blk + 1)

            # ---------- gate loop (software-pipelined) ----------
            with (
                tc.tile_pool(name="psl", bufs=2, space="PSUM") as psl,
                tc.tile_pool(name="psa", bufs=2, space="PSUM") as psa,
                tc.tile_pool(name="psx", bufs=2, space="PSUM") as psx,
                tc.tile_pool(name="gstg", bufs=3) as gstg,
                tc.tile_pool(name="gstg2", bufs=3) as gstg2,
                tc.tile_pool(name="ascp", bufs=3) as ascp,
            ):
                it_t = {}
                asc_t = {}

                def stage_logits(b):
                    embT_b = gstg.tile([D, N], F16, tag="embT_b")
                    bh, bl = divmod(b, BQ)
                    nc.sync.dma_start(
                        embT_b[:].rearrange("d (q n) -> d q n", n=NL),
                        d_e_out[bh][:, :, bl * NL: (bl + 1) * NL]
                        .rearrange("q d n -> d q n"),
                    )
                    it_b = gstg2.tile([128, 8, C], F16, tag="it_b")
                    nc.gpsimd.dma_start(
                        it_b[:],
                        inp16[b].rearrange("(q m) d -> m q d", m=128),
                    )
                    it_t[b] = it_b
                    ps_l = psl.tile([NL, N], F32, tag="ps_l")  # 2 banks
                    lhs = embT_loc[:, b * NL: (b + 1) * NL]
                    nc.tensor.matmul(ps_l[:, 0:512], lhs, embT_b[:, 0:512],
                                     start=True, stop=True)
                    nc.tensor.matmul(ps_l[:, 512:1024], lhs, embT_b[:, 512:1024],
                                     start=True, stop=True)
                    # exp with accumulated row sums
                    asc = ascp.tile([NL, N], F16, tag="asc")
                    s_col = gstg.tile([NL, 1], F32, tag="s_col")
                    nc.scalar.activation(asc[:], ps_l[:], AF.Exp,
                                         bias=neg64_col[:], accum_out=s_col[:])
                    rinv = gstg.tile([NL, 1], F32, tag="rinv")
                    nc.vector.reciprocal(rinv[:], s_col[:])
                    nc.vector.tensor_scalar_mul(asc[:], asc[:], rinv[:])
                    asc_t[b] = asc

                def stage_transpose(b):
                    asc = asc_t.pop(b)
                    ps_at = psa.tile([128, N], F16, tag="ps_at")  # 1 bank
                    for q in range(8):
                        nc.tensor.transpose(
                            ps_at[:, q * NL: (q + 1) * NL],
                            asc[:, q * NL: (q + 1) * NL],
                            ident16[:],
                        )
                    dst = A_T[:, b * 8 * NL: (b + 1) * 8 * NL]
                    if b % 2 == 0:
                        nc.vector.tensor_copy(dst, ps_at[:])
                    else:
                        nc.scalar.activation(dst, ps_at[:], AF.Copy)

                def stage_xg2(b):
                    it_b = it_t.pop(b)
                    ps_x2 = psx.tile([C, NL], F32, tag="ps_x2")
                    for q in range(8):
                        nc.tensor.matmul(
                            ps_x2[:],
                            it_b[:, q, :],
                            A_T[:, (b * 8 + q) * NL: (b * 8 + q + 1) * NL],
                            start=(q == 0), stop=(q == 7),
                        )
                    dst = (xg2T[:].rearrange("c (n b) -> c n b", b=B)[:, :, b])
                    if b % 2 == 1:
                        nc.vector.tensor_copy(dst, ps_x2[:])
                    else:
                        nc.scalar.activation(dst, ps_x2[:], AF.Copy)

                for bb in range(B + 2):
                    if bb < B:
                        stage_logits(bb)
                    if 1 <= bb < B + 1:
                        stage_transpose(bb - 1)
                    if 2 <= bb:
                        stage_xg2(bb - 2)

            # ---------- gate out-mm ----------
            def out_mm(n_o, src_k, wk_stride, act_fn, bias_rep, dest, gper):
                # gper groups per 512-col psum slab
                with (
                    tc.tile_pool(name="pso", bufs=2, space="PSUM") as pso,
                    tc.tile_pool(name="ostg", bufs=2) as ostg,
                ):
                    for gg in range(NG // gper):
                        ps_o = pso.tile([128, gper * n_o], F32, tag="ps_o")
                        for g4 in range(gper):
                            g = gg * gper + g4
                            for jj in range(4):
                                n_ = g * 4 + jj
                                for ki, src in src_k:
                                    rhs = (
                                        wslab[:, ki * wk_stride + g * (4 * n_o):
                                              ki * wk_stride + (g + 1) * (4 * n_o)]
                                        .rearrange("c (o jj) -> c o jj", jj=4)
                                        [:, :, jj]
                                    )
                                    nc.tensor.matmul(
                                        ps_o[32 * jj: 32 * jj + 32,
                                             g4 * n_o: (g4 + 1) * n_o],
                                        src[:, n_ * B: (n_ + 1) * B],
                                        rhs,
                                        start=(ki == 0), stop=(ki == 1),
                                        tile_position=(0, 32 * jj),
                                    )
                        zt = ostg.tile([128, gper * n_o], F32, tag="zt")
                        z3 = zt[:].rearrange("p (g o) -> p g o", o=n_o)
                        a0, a1 = bass.broadcast_tensor_aps(
                            ps_o[:].rearrange("p (g o) -> p g o", o=n_o),
                            bias_rep[:].unsqueeze(1),
                        )
                        nc.vector.tensor_tensor(z3, a0, a1, ALU.add)
                        nc.scalar.activation(
                            dest[:, gg * gper * n_o: (gg + 1) * gper * n_o],
                            zt[:], act_fn)

            out_mm(OG, ((0, inpT), (1, xg2T)), NG * 4 * OG, AF.Sigmoid,
                   biasg_rep, zr_sb, 4)

            # zs = z * state (one strided op)
            nc.vector.tensor_mul(
                zs_grp[:].rearrange("p (g o) -> p g o", o=DO),
                zr_sb[:].rearrange("p (g o) -> p g o", o=OG)[:, :, 0:DO],
                state_grp[:].rearrange("p (g o) -> p g o", o=DO),
            )

            # zs -> dram rows [node | (b,o)] and AG#2 in two batch halves
            for h in range(2):
                for jj in range(4):
                    nc.gpsimd.dma_start(
                        d_z_in[h][:]
                        .rearrange("(g jj) (b o) -> jj b g o", jj=4, o=DO)[jj],
                        zs_grp[32 * jj + BH * h: 32 * jj + BH * h + BH, :]
                        .rearrange("b (g o) -> b g o", o=DO),
                    )
                nc.gpsimd.collective_compute(
                    "AllGather", ALU.bypass,
                    replica_groups=[list(range(NCORES))],
                    ins=[d_z_in[h].opt()], outs=[d_z_out[h].opt()],
                )

            # ---------- overlap AG#2: wgen upd + local zs transposes ----------
            with (
                tc.tile_pool(name="pswu", bufs=4, space="PSUM") as pswu,
                tc.tile_pool(name="pwstgu", bufs=2) as pwstgu,
            ):
                wstgu = None
                for blk in range(2 * OU // 8):
                    ku, ou = divmod(blk, OU // 8)
                    pw = wgen_pw_dma(pu16, pwstgu, ku, ou * 8)
                    wgen_chunk(OU, pswu, wstgu, pw, 0, ku, ou * 8, blk)
                    wgen_chunk(OU, pswu, wstgu, pw, 1, ku, ou * 8 + 4, blk + 1)

            # inpT rows 64:128 := (z*state)^T n-major (gate out-mm done)
            with tc.tile_pool(name="psz", bufs=2, space="PSUM") as psz:
                for g8 in range(4):
                    ps_zt = psz.tile([DO, 8 * 128], F16, tag="ps_zt")
                    for i in range(8):
                        g = g8 * 8 + i
                        nc.tensor.transpose(
                            ps_zt[:, i * 128: (i + 1) * 128],
                            zs_grp[:, g * DO: (g + 1) * DO],
                            ident16[:],
                        )
                    nc.vector.tensor_copy(
                        inpT[DI:C, g8 * 8 * 128: (g8 + 1) * 8 * 128],
                        ps_zt[:],
                    )

            # ---------- PV upd: xg2T rows 64:128 := (A @ zs)^T ----------
            with (
                tc.tile_pool(name="psp", bufs=2, space="PSUM") as psp,
                tc.tile_pool(name="pvstg", bufs=3) as pvstg,
            ):
                npair = B // 2 if COLTILE_PV else B
                for p in range(npair):
                    if COLTILE_PV:
                        bs = (2 * p, 2 * p + 1)
                    else:
                        bs = (p,)
                    zsb = pvstg.tile([128, 8, len(bs), DO], F16, tag="zsb")
                    bh, bl = divmod(bs[0], BH)
                    nc.sync.dma_start(
                        zsb[:],
                        d_z_out[bh][:]
                        .rearrange("q m (b o) -> m q b o", o=DO)
                        [:, :, bl: bl + len(bs), :],
                    )
                    ps_pv = psp.tile([128, NL], F32, tag="ps_pv")
                    for q in range(8):
                        for i, b in enumerate(bs):
                            nc.tensor.matmul(
                                ps_pv[64 * i: 64 * i + 64, :],
                                zsb[:, q, i, :],
                                A_T[:, (b * 8 + q) * NL: (b * 8 + q + 1) * NL],
                                start=(q == 0), stop=(q == 7),
                                tile_position=(0, 64 * i) if COLTILE_PV else None,
                            )
                    for i, b in enumerate(bs):
                        dst = (xg2T[DI:C, :]
                               .rearrange("c (n b) -> c n b", b=B)[:, :, b])
                        if i % 2 == 0:
                            nc.vector.tensor_copy(
                                dst, ps_pv[64 * i: 64 * i + 64, :])
                        else:
                            nc.scalar.activation(
                                dst, ps_pv[64 * i: 64 * i + 64, :], AF.Copy)

            # ---------- upd out-mm ----------
            out_mm(OU, ((0, inpT), (1, xg2T)), NG * 4 * OU, AF.Tanh,
                   biasu_rep, hc_sb, 8)

            # ---------- combine: h = r*(state - hc) + hc (zs_grp as scratch) ----------
            nc.vector.tensor_sub(zs_grp[:], state_grp[:], hc_sb[:])
            nc.vector.tensor_mul(
                zs_grp[:].rearrange("p (g o) -> p g o", o=DO),
                zs_grp[:].rearrange("p (g o) -> p g o", o=DO),
                zr_sb[:].rearrange("p (g o) -> p g o", o=OG)[:, :, DO:OG],
            )
            nc.vector.tensor_add(zs_grp[:], zs_grp[:], hc_sb[:])
            nc.gpsimd.dma_start(h_out[:], zs_grp[:])

    nc.finalize()
    return nc


def _get_nc():
    if "nc" not in _CACHE:
        _CACHE["nc"] = _build()
    return _CACHE["nc"]


def kernel(x, state, node_emb, time_emb, gate_w, gate_b, gate_gamma, gate_beta,
           upd_w, upd_b, upd_gamma, upd_beta):
    global LAST_RESULT
    x = np.asarray(x, np.float32)
    state = np.asarray(state, np.float32)
    node_emb = np.asarray(node_emb, np.float32)
    time_emb = np.asarray(time_emb, np.float32)
    gate_w = np.asarray(gate_w, np.float32)
    gate_b = np.asarray(gate_b, np.float32)
    upd_w = np.asarray(upd_w, np.float32)
    upd_b = np.asarray(upd_b, np.float32)

    shared = (
        np.array_equal(np.asarray(gate_gamma), np.ones(D, np.float32))
        and np.array_equal(np.asarray(upd_gamma), np.ones(D, np.float32))
        and np.array_equal(np.asarray(gate_beta), np.zeros(D, np.float32))
        and np.array_equal(np.asarray(upd_beta), np.zeros(D, np.float32))
    )
    if not shared:
        return _np_reference(x, state, node_emb, time_emb, gate_w, gate_b,
                             gate_gamma, gate_beta, upd_w, upd_b, upd_gamma,
                             upd_beta)

    if os.environ.get("BASS_TRACE"):
        _install_prof_shim()

    from concourse.bass_utils import run_bass_kernel_spmd

    nc = _get_nc()

    inp16 = np.concatenate([x, state], axis=-1).astype(np.float16)  # [B,N,C]
    xTn = np.ascontiguousarray(x.transpose(2, 1, 0)).astype(np.float16)   # [DI,N,B]
    stTn = np.ascontiguousarray(state.transpose(2, 1, 0)).astype(np.float16)
    neT16 = np.ascontiguousarray(node_emb.T).astype(np.float16)
    teT16 = np.ascontiguousarray(time_emb.T).astype(np.float16)
    pg16 = np.ascontiguousarray(gate_w.transpose(1, 3, 0, 2)).astype(np.float16)
    pu16 = np.ascontiguousarray(upd_w.transpose(1, 3, 0, 2)).astype(np.float16)
    te_row = np.ascontiguousarray(time_emb.reshape(1, B * D))

    in_maps = []
    for c in range(NCORES):
        nlo = c * NL
        in_maps.append({
            "ne_f32": np.ascontiguousarray(node_emb[nlo: nlo + NL]),
            "neT16": np.ascontiguousarray(neT16[:, nlo: nlo + NL]),
            "te_f32": te_row,
            "teT16": teT16,
            "inp16": inp16,
            "xTn16": np.ascontiguousarray(xTn[:, nlo: nlo + NL, :]),
            "stTn16": np.ascontiguousarray(stTn[:, nlo: nlo + NL, :]),
            "st_loc": np.ascontiguousarray(
                state[:, nlo: nlo + NL, :]).astype(np.float16),
            "pg16": pg16,
            "pu16": pu16,
            "gb16": gate_b.astype(np.float16),
            "ub16": upd_b.astype(np.float16),
        })

    res = run_bass_kernel_spmd(
        nc, in_maps, list(range(NCORES)),
        trace=bool(os.environ.get("BASS_TRACE")),
    )
    LAST_RESULT = res

    h = np.empty((B, N, DO), np.float32)
    for c in range(NCORES):
        ho = res.results[c]["h_out"].astype(np.float32).reshape(4, 32, NG, DO)
        h[:, c * NL: (c + 1) * NL, :] = (
            ho.transpose(1, 2, 0, 3).reshape(B, NL, DO)
        )
    return h
